# revision 31
# baseline (speedup 1.0000x reference)
"""AttentionDTI forward on 8 Trainium2 NeuronCores (Bass/Tile), data-parallel.

Layout strategy (per core, batch shard b=256):
  - channels live on SBUF partitions everywhere; positions/samples on free dims
  - embedding lookup fused into conv1: G_k = emb @ dw1[:,:,k].T  (host prep),
    device builds one-hot [65, S*100] from int32 drug ids (broadcast DMA +
    one is_equal tensor_scalar op) and matmuls against G_k
  - conv2/conv3 = shifted matmuls accumulated in PSUM over taps/Cin chunks
  - attention computed channel-major: da = Wda@dc, s = relu(da + fa_bcast),
    A = Watt@s; comp/feat scales via ScalarE sigmoid straight from PSUM
  - both attention matmuls (Wda, Watt) run as fp8e4 DoubleRow (2 contraction
    chunks per instruction): their outputs feed sigmoids / are re-added to
    full-precision paths, so e4m3 quantization is loss-free here (verified
    against an fp32 emulation); dc gets a separate fp8 copy evacuated from
    the same PSUM so the bf16 dc still feeds the (0.5+comp) scale + maxpool
  - MLP (1024-1024-512-1) batched over all 256 samples at the end
Other matmul operands bf16 (PSUM accumulates f32); biases folded into ACT
ops. PSUM banks: 1 conv1 + 1 conv2/out + 6 rotating for conv3/da/attn.

Runner: a process-cached jit(shard_map(bass_exec)) executable with the
replicated weights kept device-resident (keyed by a content hash), so a
steady-state call only ships drug/feature (~860KB) over the axon tunnel
instead of ~70MB of weights.
"""

import hashlib
import sys
import time
from types import SimpleNamespace

if "/opt/trn_rl_repo" not in sys.path:
    sys.path.insert(0, "/opt/trn_rl_repo")

import numpy as np
import ml_dtypes

import concourse.bass as bass
import concourse.tile as tile
from concourse import mybir

BF16 = mybir.dt.bfloat16
F8 = mybir.dt.float8e4
F32 = mybir.dt.float32
I32 = mybir.dt.int32
bf16 = ml_dtypes.bfloat16
f8e4 = ml_dtypes.float8_e4m3  # TRN FP8_EXP4-compatible (max ±240)

N_CORES = 8
B = 2048
B_CORE = B // N_CORES
LD = 100
L1, L2, L3 = 97, 92, 85  # lengths after K=4,6,8 valid convs
S = 5  # samples per tile (S*L1 = 485 <= 512 psum bank; matmul FD>512
#        fails the walrus ISA check -- output must fit one PSUM bank)
PAD8 = 432  # fp8 chunk pitch: >= S*L3=425, multiple of 16 (DoubleRow
#             weight-pair stride constraint)

AF = mybir.ActivationFunctionType
ALU = mybir.AluOpType


# --------------------------------------------------------------------------
# walrus's CTRL codegen handles at most 2 sem waits on one instruction; the
# Tile tail drain can carry many. Split them across single-wait SP nops.
def _patched_drain_and_barrier(self, tick_clock, wait_clock):
    from concourse.tile import ScopedClock

    nc = self.nc
    probe = nc.sync.nop()
    wait_clock.add_sem_waits(probe.ins, ScopedClock({None: tick_clock.global_clock}))
    si = probe.ins.sync_info
    waits = list(si.on_wait) if si is not None else []
    if si is not None:
        probe.ins.sync_info = mybir.SyncInfo(
            on_update=list(si.on_update), on_wait=waits[:1]
        )
    for w in waits[1:]:
        extra = nc.sync.nop()
        extra.ins.sync_info = mybir.SyncInfo(on_update=[], on_wait=[w])
    nc.sync.drain()
    nc.all_engine_barrier()
    popped = nc._tile_sem_poison_stack.pop()
    assert popped is self._sem_poison
    nc.clear_and_free_semaphores(list(self.sems.allocated().values()))
    nc.all_engine_barrier()


tile.TileContext._drain_and_barrier = _patched_drain_and_barrier


# Same walrus limit applies to every engine instruction, and Tile's sem
# assignment can put 3+ waits on one op. Rewrite the serialized BIR: any
# instruction with >2 waits gets same-engine NoOps in front carrying the
# surplus (waits are AND conditions, so hoisting preserves semantics).
_MAX_WAITS = 1
# guard against double-wrapping if this module is ever reloaded
if not getattr(bass.Bass.to_json_bytes, "_wait_split_patched", False):
    _orig_to_json_bytes = bass.Bass.to_json_bytes


def _split_waits_to_json_bytes(self, *a, **k):
    import json as _json

    raw = _orig_to_json_bytes(self, *a, **k)
    j = _json.loads(raw)
    ctr = 0
    changed = False
    for f in j.get("functions", []):
        for bb in f.get("blocks", []):
            out = []
            for ins in bb.get("instructions", []):
                si = ins.get("sync_info")
                waits = (si or {}).get("on_wait", [])
                if len(waits) > _MAX_WAITS:
                    changed = True
                    extra, keep = waits[:-_MAX_WAITS], waits[-_MAX_WAITS:]
                    for i in range(0, len(extra), _MAX_WAITS):
                        ctr += 1
                        out.append({
                            "debug": ins.get("debug"),
                            "engine": ins["engine"],
                            "ins": [],
                            "name": f"I-wsplit-{ctr}",
                            "opcode": "NoOp",
                            "outs": [],
                            "sync_info": {
                                "on_update": [],
                                "on_wait": extra[i : i + _MAX_WAITS],
                            },
                        })
                    si["on_wait"] = keep
                out.append(ins)
            bb["instructions"] = out
    if not changed:
        return raw
    return _json.dumps(j).encode()


if not getattr(bass.Bass.to_json_bytes, "_wait_split_patched", False):
    _split_waits_to_json_bytes._wait_split_patched = True
    bass.Bass.to_json_bytes = _split_waits_to_json_bytes
# --------------------------------------------------------------------------


def _bcast_free(ap, n):
    """Append an innermost stride-0 free dim of size n (broadcast read)."""
    return bass.AP(tensor=ap.tensor, offset=ap.offset, ap=list(ap.ap) + [[0, n]])


def _v3(t8, mc, st):
    """[128, st, L3] view of chunk mc of a flat [128, 4, PAD8] fp8 tile
    (the first S*L3 elements of each chunk are contiguous sample blocks)."""
    a = t8[:, mc]
    return bass.AP(
        tensor=a.tensor, offset=a.offset,
        ap=[list(a.ap[0]), [L3, st], [1, L3]],
    )


BIAS_COLS = [
    ("db1", 1), ("db2", 2), ("db3", 4), ("fb1", 1), ("fb2", 2), ("fb3", 4),
    ("fbias", 4), ("batt", 4), ("b1", 8), ("b2", 8), ("b3", 4), ("bo", 1),
]
N_BIAS = sum(n for _, n in BIAS_COLS)  # 43
COL = {}
_c = 0
for _name, _n in BIAS_COLS:
    COL[_name] = _c
    _c += _n


def build_nc(b_core=B_CORE, reps=1):
    """Build the Bass program. With reps>1 the whole kernel body (including
    the weight/const DMAs from DRAM) is emitted reps times back-to-back —
    used only for timing, so device execution time can be measured as the
    slope between two reps values, independent of host dispatch overhead.
    (This walrus build rejects Tile For_i loop instructions, so the reps
    are unrolled.)"""
    nc = bass.Bass()
    dp = nc.declare_dram_parameter

    drug = dp("drug", [b_core, LD], I32, isOutput=False)
    feature = dp("feature", [b_core, 7], F32, isOutput=False)
    iota65 = dp("iota65", [128, 1], F32, isOutput=False)
    g_w = dp("g_w", [128, 2, 128], BF16, isOutput=False)
    w2 = dp("w2", [128, 6, 256], BF16, isOutput=False)
    w3 = dp("w3", [128, 2, 8, 512], BF16, isOutput=False)
    wda = dp("wda", [128, 4, 512], F8, isOutput=False)
    watt = dp("watt", [128, 4, 512], F8, isOutput=False)
    wfa = dp("wfa", [128, 4, 512], BF16, isOutput=False)
    fw1 = dp("fw1", [7, 128], BF16, isOutput=False)
    fw2 = dp("fw2", [128, 256], BF16, isOutput=False)
    fw3 = dp("fw3", [128, 2, 512], BF16, isOutput=False)
    w1m = dp("w1m", [128, 8, 1024], BF16, isOutput=False)
    w2m = dp("w2m", [128, 8, 1024], BF16, isOutput=False)
    w3m = dp("w3m", [128, 8, 512], BF16, isOutput=False)
    wom = dp("wom", [128, 4], BF16, isOutput=False)
    bias = dp("bias", [128, N_BIAS], F32, isOutput=False)
    out_p = dp("out", [1, b_core], F32, isOutput=True)

    mm = nc.tensor.matmul
    act = nc.scalar.activation

    with tile.TileContext(nc) as tc:
        with (
            tc.tile_pool(name="const", bufs=1) as const,
            tc.tile_pool(name="keep", bufs=1) as keep,
            tc.tile_pool(name="work", bufs=2) as work,
            tc.tile_pool(name="ps1", bufs=1, space="PSUM") as ps1,
            tc.tile_pool(name="ps2", bufs=1, space="PSUM") as ps2,
            tc.tile_pool(name="psb", bufs=6, space="PSUM") as psb,
        ):
          def _whole_body():
            # ---------------- constants (needed-first DMA order) -----------
            iota_sb = const.tile([128, 1], F32)
            nc.sync.dma_start(out=iota_sb, in_=iota65[:, :])
            g_sb = const.tile([128, 2, 128], BF16)
            nc.sync.dma_start(out=g_sb, in_=g_w[:, :, :])
            bias_sb = const.tile([128, N_BIAS], F32)
            nc.sync.dma_start(out=bias_sb, in_=bias[:, :])
            w2_sb = const.tile([128, 6, 256], BF16)
            nc.sync.dma_start(out=w2_sb, in_=w2[:, :, :])
            fw1_sb = const.tile([7, 128], BF16)
            nc.sync.dma_start(out=fw1_sb, in_=fw1[:, :])
            fw2_sb = const.tile([128, 256], BF16)
            nc.sync.dma_start(out=fw2_sb, in_=fw2[:, :])
            fw3_sb = const.tile([128, 2, 512], BF16)
            nc.sync.dma_start(out=fw3_sb, in_=fw3[:, :, :])
            f_sb = keep.tile([7, b_core], F32)
            feat_ap = feature[:, :]
            fT = bass.AP(
                tensor=feat_ap.tensor, offset=feat_ap.offset, ap=[[1, 7], [7, b_core]]
            )
            nc.sync.dma_start(out=f_sb, in_=fT)
            wfa_sb = const.tile([128, 4, 512], BF16)
            nc.sync.dma_start(out=wfa_sb, in_=wfa[:, :, :])
            w3_sb = const.tile([128, 2, 8, 512], BF16)
            nc.sync.dma_start(out=w3_sb, in_=w3[:, :, :, :])
            wda_sb = const.tile([128, 4, 512], F8)
            nc.sync.dma_start(out=wda_sb, in_=wda[:, :, :])
            watt_sb = const.tile([128, 4, 512], F8)
            nc.sync.dma_start(out=watt_sb, in_=watt[:, :, :])
            w1m_sb = const.tile([128, 8, 1024], BF16)
            nc.sync.dma_start(out=w1m_sb, in_=w1m[:, :, :])
            w2m_sb = const.tile([128, 8, 1024], BF16)
            nc.sync.dma_start(out=w2m_sb, in_=w2m[:, :, :])
            w3m_sb = const.tile([128, 8, 512], BF16)
            nc.sync.dma_start(out=w3m_sb, in_=w3m[:, :, :])
            wom_sb = const.tile([128, 4], BF16)
            nc.sync.dma_start(out=wom_sb, in_=wom[:, :])

            def bcol(name, i=0):
                return bias_sb[:, COL[name] + i : COL[name] + i + 1]

            # accumulators for the MLP input
            vd_sb = keep.tile([128, 4, b_core], BF16)
            vf_sb = keep.tile([128, 4, b_core], BF16)

            # ---------------- main per-tile loop (software-pipelined) ------
            # PE program order per steady iteration:
            #   [conv1+conv2](t+1)  [da](t)  [conv3](t+1)  [A](t)
            # so every stage consumes results produced >= one full stage
            # earlier and PE never waits on an evacuation.
            n_tiles = (b_core + S - 1) // S

            def emit_feature_path():
                fb_sb = keep.tile([7, b_core], BF16)
                nc.vector.tensor_copy(out=fb_sb, in_=f_sb)

                psf = ps1.tile([128, b_core], F32, tag="c1")
                mm(psf, lhsT=fw1_sb, rhs=fb_sb, start=True, stop=True)
                h1f = keep.tile([128, b_core], BF16)
                act(out=h1f, in_=psf, func=AF.Relu, bias=bcol("fb1"), scale=1.0)

                h2f = keep.tile([128, 2, b_core], BF16)
                for mc in range(2):
                    psf2 = ps1.tile([128, b_core], F32, tag="c1")
                    mm(psf2, lhsT=fw2_sb[:, mc * 128 : (mc + 1) * 128], rhs=h1f,
                       start=True, stop=True)
                    act(out=h2f[:, mc], in_=psf2, func=AF.Relu, bias=bcol("fb2", mc),
                        scale=1.0)

                fnn_sb = keep.tile([128, 4, b_core], BF16)
                for mc in range(4):
                    psf3 = ps1.tile([128, b_core], F32, tag="c1")
                    for kc in range(2):
                        mm(psf3, lhsT=fw3_sb[:, kc, mc * 128 : (mc + 1) * 128],
                           rhs=h2f[:, kc], start=(kc == 0), stop=(kc == 1))
                    act(out=fnn_sb[:, mc], in_=psf3, func=AF.Relu,
                        bias=bcol("fb3", mc), scale=1.0)

                # fa = Wfa @ featureNN + (bda + bfa)   [512, b] f32, kept
                fa_sb = keep.tile([128, 4, b_core], F32)
                for mc in range(4):
                    psfa = ps1.tile([128, b_core], F32, tag="c1")
                    for kc in range(4):
                        mm(psfa, lhsT=wfa_sb[:, kc, mc * 128 : (mc + 1) * 128],
                           rhs=fnn_sb[:, kc], start=(kc == 0), stop=(kc == 3))
                    nc.vector.tensor_scalar_add(
                        out=fa_sb[:, mc], in0=psfa, scalar1=bcol("fbias", mc)
                    )
                return fnn_sb, fa_sb

            def emit_front(t):
                """drug DMA + packed one-hot + conv1 + conv2 -> h2 tile.
                One-hot rows 0-63 = onehot(v=1..64), rows 64-127 = the same
                shifted left one position (vocab row 0 of the emb-fused conv1
                weight is zero, so it is dropped); conv1 then packs two taps
                into each 128-contract matmul."""
                b0 = t * S
                st = min(S, b_core - b0)
                drug_bc = work.tile([128, S, LD], I32, tag="drug",
                                    name=f"drug_bc{t}")
                drug_ap = drug[:, :]
                src = bass.AP(tensor=drug_ap.tensor,
                              offset=drug_ap.offset + b0 * LD,
                              ap=[[0, 128], [LD, st], [1, LD]])
                nc.gpsimd.dma_start(out=drug_bc[:, :st], in_=src)
                oh = work.tile([128, S, LD], BF16, tag="oh", name=f"oh{t}")
                nc.vector.tensor_scalar(
                    out=oh[0:64, :st], in0=drug_bc[0:64, :st], scalar1=iota_sb[0:64],
                    scalar2=None, op0=ALU.is_equal,
                )
                nc.vector.tensor_scalar(
                    out=oh[64:128, :st, 0 : LD - 1],
                    in0=drug_bc[64:128, :st, 1:LD], scalar1=iota_sb[64:128],
                    scalar2=None, op0=ALU.is_equal,
                )

                pc1 = ps1.tile([128, S, L1], F32, tag="c1", name=f"pc1_{t}")
                for j in range(2):
                    mm(pc1[:, :st], lhsT=g_sb[:, j], rhs=oh[:, :st, 2 * j : 2 * j + L1],
                       start=(j == 0), stop=(j == 1))
                h1 = work.tile([128, S, L1], BF16, tag="h1", name=f"h1_{t}")
                act(out=h1[:, :st], in_=pc1[:, :st], func=AF.Relu, bias=bcol("db1"),
                    scale=1.0)

                h2 = work.tile([128, 2, S, L2], BF16, tag="h2", name=f"h2_{t}")
                for mc in range(2):
                    pc2 = ps2.tile([128, S, L2], F32, tag="c2", name=f"pc2_{t}_{mc}")
                    for k in range(6):
                        mm(pc2[:, :st], lhsT=w2_sb[:, k, mc * 128 : (mc + 1) * 128],
                           rhs=h1[:, :st, k : k + L2], start=(k == 0), stop=(k == 5))
                    act(out=h2[:, mc, :st], in_=pc2[:, :st], func=AF.Relu,
                        bias=bcol("db2", mc), scale=1.0)
                return h2

            def emit_conv3(t, h2):
                b0 = t * S
                st = min(S, b_core - b0)
                dc = work.tile([128, 4, S, L3], BF16, tag="dc", name=f"dc{t}")
                dc8 = work.tile([128, 4, PAD8], F8, tag="dc8", name=f"dc8_{t}")
                for mc in range(4):
                    pc3 = psb.tile([128, S, L3], F32, tag="big", name=f"pc3_{t}_{mc}")
                    i = 0
                    for kc in range(2):
                        for k in range(8):
                            mm(pc3[:, :st],
                               lhsT=w3_sb[:, kc, k, mc * 128 : (mc + 1) * 128],
                               rhs=h2[:, kc, :st, k : k + L3],
                               start=(i == 0), stop=(i == 15))
                            i += 1
                    act(out=dc[:, mc, :st], in_=pc3[:, :st], func=AF.Relu,
                        bias=bcol("db3", mc), scale=1.0)
                    # second evacuation of the same PSUM as an fp8 copy that
                    # only feeds the Wda matmul (DoubleRow needs fp8 operands;
                    # dc itself stays bf16 for the (0.5+comp) scaling + maxpool)
                    act(out=_v3(dc8, mc, st), in_=pc3[:, :st], func=AF.Relu,
                        bias=bcol("db3", mc), scale=1.0)
                return dc, dc8

            def emit_da(t, dc8):
                b0 = t * S
                st = min(S, b_core - b0)
                s8 = work.tile([128, 4, PAD8], F8, tag="s8", name=f"s8_{t}")
                for mc in range(4):
                    pda = psb.tile([128, S, L3], F32, tag="big", name=f"pda_{t}_{mc}")
                    for j in range(2):
                        mm(pda[:, :st],
                           lhsT=wda_sb[:, 2 * j : 2 * j + 2, mc * 128 : (mc + 1) * 128],
                           rhs=dc8[:, 2 * j : 2 * j + 2, : st * L3],
                           perf_mode=mybir.MatmulPerfMode.DoubleRow,
                           start=(j == 0), stop=(j == 1))
                    fa_b = _bcast_free(fa_sb[:, mc, b0 : b0 + st], L3)
                    nc.vector.tensor_tensor(
                        out=_v3(s8, mc, st), in0=pda[:, :st], in1=fa_b, op=ALU.add
                    )
                    nc.vector.tensor_scalar_max(
                        out=_v3(s8, mc, st), in0=_v3(s8, mc, st), scalar1=0.0
                    )
                return s8

            def emit_attn(t, dc, s8):
                b0 = t * S
                st = min(S, b_core - b0)
                dcs = work.tile([128, 4, S, L3], BF16, tag="dcs", name=f"dcs{t}")
                for mc in range(4):
                    pA = psb.tile([128, S, L3], F32, tag="big", name=f"pA_{t}_{mc}")
                    for j in range(2):
                        mm(pA[:, :st],
                           lhsT=watt_sb[:, 2 * j : 2 * j + 2, mc * 128 : (mc + 1) * 128],
                           rhs=s8[:, 2 * j : 2 * j + 2, : st * L3],
                           perf_mode=mybir.MatmulPerfMode.DoubleRow,
                           start=(j == 0), stop=(j == 1))
                    u = work.tile([128, S, L3], BF16, tag="u", name=f"u{t}_{mc}")
                    act(out=u[:, :st], in_=pA[:, :st], func=AF.Sigmoid,
                        bias=bcol("batt", mc), scale=1.0)
                    asum = work.tile([128, S], F32, tag="asum", name=f"as{t}_{mc}")
                    nc.vector.tensor_reduce(
                        out=asum[:, :st], in_=pA[:, :st], axis=mybir.AxisListType.X,
                        op=ALU.add,
                    )
                    fsc = work.tile([128, S], F32, tag="fsc", name=f"fs{t}_{mc}")
                    act(out=fsc[:, :st], in_=asum[:, :st], func=AF.Sigmoid,
                        bias=bcol("batt", mc), scale=1.0 / L3)
                    nc.vector.scalar_tensor_tensor(
                        out=dcs[:, mc, :st], in0=u[:, :st], scalar=0.5,
                        in1=dc[:, mc, :st], op0=ALU.add, op1=ALU.mult,
                    )
                    nc.vector.tensor_reduce(
                        out=vd_sb[:, mc, b0 : b0 + st], in_=dcs[:, mc, :st],
                        axis=mybir.AxisListType.X, op=ALU.max,
                    )
                    nc.vector.scalar_tensor_tensor(
                        out=vf_sb[:, mc, b0 : b0 + st], in0=fsc[:, :st], scalar=0.5,
                        in1=fnn_sb[:, mc, b0 : b0 + st], op0=ALU.add, op1=ALU.mult,
                    )

            h2_cur = emit_front(0)
            fnn_sb, fa_sb = emit_feature_path()
            dc_cur, dc8_cur = emit_conv3(0, h2_cur)
            for t in range(n_tiles):
                h2_next = emit_front(t + 1) if t + 1 < n_tiles else None
                s_cur = emit_da(t, dc8_cur)
                dc_next, dc8_next = (
                    emit_conv3(t + 1, h2_next) if h2_next is not None else (None, None)
                )
                emit_attn(t, dc_cur, s_cur)
                dc_cur, dc8_cur = dc_next, dc8_next

            # ------- MLP over the shard, two batch halves interleaved -------
            def pair(kc):
                return vd_sb[:, kc] if kc < 4 else vf_sb[:, kc - 4]

            def leaky_evac(dst, psm, bias_ap, hb, i):
                z = work.tile([128, b_core // 2], F32, tag="z", name=f"z{hb}_{i}")
                act(out=z, in_=psm, func=AF.Identity, bias=bias_ap, scale=1.0)
                nc.vector.scalar_tensor_tensor(
                    out=dst, in0=z, scalar=0.01, in1=z, op0=ALU.mult, op1=ALU.max
                )

            HB = b_core // 2
            hm1 = keep.tile([128, 8, b_core], BF16)
            hm2 = keep.tile([128, 8, b_core], BF16)
            hm3 = keep.tile([128, 4, b_core], BF16)

            def mlp_layer(wsb, n_mc, rhs_of, dst, bname, hb):
                lo = hb * HB
                sl = slice(lo, lo + HB)
                for mc in range(n_mc):
                    pp, tg = (ps1, "c1") if mc % 2 == 0 else (ps2, "c2")
                    psm = pp.tile([128, HB], F32, tag=tg,
                                  name=f"psm_{bname}_{hb}_{mc}")
                    for kc in range(8):
                        mm(psm, lhsT=wsb[:, kc, mc * 128 : (mc + 1) * 128],
                           rhs=rhs_of(kc)[:, sl], start=(kc == 0), stop=(kc == 7))
                    leaky_evac(dst[:, mc, sl], psm, bcol(bname, mc), hb,
                               f"{bname}{mc}")

            for hb in range(2):
                mlp_layer(w1m_sb, 8, pair, hm1, "b1", hb)
            for hb in range(2):
                mlp_layer(w2m_sb, 8, lambda kc: hm1[:, kc], hm2, "b2", hb)
            for hb in range(2):
                mlp_layer(w3m_sb, 4, lambda kc: hm2[:, kc], hm3, "b3", hb)

            pso = ps2.tile([1, b_core], F32, tag="c2")
            for kc in range(4):
                mm(pso, lhsT=wom_sb[:, kc : kc + 1], rhs=hm3[:, kc],
                   start=(kc == 0), stop=(kc == 3))
            o_sb = work.tile([1, b_core], F32, tag="o")
            nc.vector.tensor_scalar_add(
                out=o_sb, in0=pso, scalar1=bias_sb[0:1, COL["bo"] : COL["bo"] + 1]
            )
            nc.gpsimd.dma_start(out=out_p[:, :], in_=o_sb)

          for _ in range(reps):
              _whole_body()

    return nc


def _prep_weights(inp):
    f32 = np.float32

    def t(x):
        return np.ascontiguousarray(x)

    emb = np.asarray(inp["emb"], f32)
    dw1 = np.asarray(inp["dw1"], f32)
    dw2 = np.asarray(inp["dw2"], f32)
    dw3 = np.asarray(inp["dw3"], f32)
    G = np.stack([emb @ dw1[:, :, k].T for k in range(4)], 0)  # [4, 65, 128]

    w = {}
    iota2 = np.concatenate([np.arange(1, 65), np.arange(1, 65)]).astype(np.float32)
    w["iota65"] = iota2.reshape(128, 1)
    g2 = np.zeros((128, 2, 128), np.float32)
    for j in range(2):
        g2[0:64, j] = G[2 * j][1:65]
        g2[64:128, j] = G[2 * j + 1][1:65]
    w["g_w"] = g2.astype(bf16)
    w["w2"] = t(dw2.transpose(1, 2, 0)).astype(bf16)  # [128, 6, 256]
    w["w3"] = t(
        dw3.reshape(512, 2, 128, 8).transpose(2, 1, 3, 0)
    ).astype(bf16)  # [128, 2, 8, 512]
    for nm, W, dt8 in [("wda", "Wda", f8e4), ("watt", "Watt", f8e4),
                       ("wfa", "Wfa", bf16)]:
        M = np.asarray(inp[W], f32).T  # [c, d]
        w[nm] = t(M.reshape(4, 128, 512).transpose(1, 0, 2)).astype(dt8)
    w["fw1"] = t(np.asarray(inp["fw1"], f32)[:, :, 1].T).astype(bf16)  # [7, 128]
    w["fw2"] = t(np.asarray(inp["fw2"], f32)[:, :, 1].T).astype(bf16)  # [128, 256]
    w["fw3"] = t(
        np.asarray(inp["fw3"], f32)[:, :, 1].T.reshape(2, 128, 512).transpose(1, 0, 2)
    ).astype(bf16)  # [128, 2, 512]
    w["w1m"] = t(
        np.asarray(inp["W1"], f32).T.reshape(8, 128, 1024).transpose(1, 0, 2)
    ).astype(bf16)
    w["w2m"] = t(
        np.asarray(inp["W2"], f32).T.reshape(8, 128, 1024).transpose(1, 0, 2)
    ).astype(bf16)
    w["w3m"] = t(
        np.asarray(inp["W3"], f32).T.reshape(8, 128, 512).transpose(1, 0, 2)
    ).astype(bf16)
    w["wom"] = t(np.asarray(inp["Wo"], f32).T.reshape(4, 128).T).astype(bf16)

    cols = []
    cols.append(np.asarray(inp["db1"], f32).reshape(128, 1))
    cols.append(np.asarray(inp["db2"], f32).reshape(2, 128).T)
    cols.append(np.asarray(inp["db3"], f32).reshape(4, 128).T)
    cols.append(np.asarray(inp["fb1"], f32).reshape(128, 1))
    cols.append(np.asarray(inp["fb2"], f32).reshape(2, 128).T)
    cols.append(np.asarray(inp["fb3"], f32).reshape(4, 128).T)
    fbias = np.asarray(inp["bda"], f32) + np.asarray(inp["bfa"], f32)
    cols.append(fbias.reshape(4, 128).T)
    cols.append(np.asarray(inp["batt"], f32).reshape(4, 128).T)
    cols.append(np.asarray(inp["b1"], f32).reshape(8, 128).T)
    cols.append(np.asarray(inp["b2"], f32).reshape(8, 128).T)
    cols.append(np.asarray(inp["b3"], f32).reshape(4, 128).T)
    bo_val = float(np.asarray(inp["bo"], f32).reshape(-1)[0])
    cols.append(np.full((128, 1), bo_val, f32))
    w["bias"] = np.ascontiguousarray(np.concatenate(cols, axis=1))
    assert w["bias"].shape == (128, N_BIAS)
    return w


# ---------------------------------------------------------------------------
# Runner: build the jit(shard_map(bass_exec)) executable once per process and
# keep the replicated weights device-resident across calls.
# ---------------------------------------------------------------------------

_STATES = {}  # reps -> SimpleNamespace
_WDEV_CACHE = {}  # weights content hash -> dict name -> committed jax.Array
_WDEV_ORDER = []


def _get_state(reps=1):
    if reps in _STATES:
        return _STATES[reps]

    import jax
    from jax.experimental.shard_map import shard_map
    from jax.sharding import Mesh, NamedSharding, PartitionSpec
    from concourse import bass2jax

    bass2jax.install_neuronx_cc_hook()
    nc = build_nc(B_CORE, reps=reps)
    assert nc.dbg_addr is None

    partition_name = (
        nc.partition_id_tensor.name if nc.partition_id_tensor else None
    )
    in_names, out_names, out_avals, zero_outs = [], [], [], []
    for alloc in nc.m.functions[0].allocations:
        if not isinstance(alloc, mybir.MemoryLocationSet):
            continue
        name = alloc.memorylocations[0].name
        if alloc.kind == "ExternalInput":
            if name != partition_name:
                in_names.append(name)
        elif alloc.kind == "ExternalOutput":
            shape = tuple(alloc.tensor_shape)
            dtype = mybir.dt.np(alloc.dtype)
            out_avals.append(jax.core.ShapedArray(shape, dtype))
            out_names.append(name)
            zero_outs.append(np.zeros(shape, dtype))
    n_params = len(in_names)
    n_outs = len(out_names)
    all_in_names = list(in_names) + list(out_names)
    if partition_name is not None:
        all_in_names.append(partition_name)
    # No donation: the kernel writes every element of its outputs, so the
    # zero "output seed" operands are never observed and one shared
    # device-resident zeros array can be reused across calls (donating a
    # fresh host zeros array per call costs ~1.2ms of transfer latency).

    def _body(*args):
        operands = list(args)
        if partition_name is not None:
            operands.append(bass2jax.partition_id_tensor())
        outs = bass2jax._bass_exec_p.bind(
            *operands,
            out_avals=tuple(out_avals),
            in_names=tuple(all_in_names),
            out_names=tuple(out_names),
            lowering_input_output_aliases=(),
            sim_require_finite=True,
            sim_require_nnan=True,
            nc=nc,
        )
        return tuple(outs)

    devices = jax.devices()[:N_CORES]
    assert len(devices) == N_CORES
    mesh = Mesh(np.asarray(devices), ("core",))
    sharding = NamedSharding(mesh, PartitionSpec("core"))
    sharded = jax.jit(
        shard_map(
            _body,
            mesh=mesh,
            in_specs=(PartitionSpec("core"),) * (n_params + n_outs),
            out_specs=(PartitionSpec("core"),) * n_outs,
            check_rep=False,
        ),
        keep_unused=True,
    )
    state = SimpleNamespace(
        nc=nc,
        jax=jax,
        mesh=mesh,
        sharding=sharding,
        sharded=sharded,
        in_names=in_names,
        out_names=out_names,
        zero_outs=zero_outs,
        zeros_dev=None,
    )
    _STATES[reps] = state
    return state


_WEIGHT_KEYS = [
    "emb", "dw1", "db1", "dw2", "db2", "dw3", "db3",
    "fw1", "fb1", "fw2", "fb2", "fw3", "fb3",
    "Wda", "bda", "Wfa", "bfa", "Watt", "batt",
    "W1", "b1", "W2", "b2", "W3", "b3", "Wo", "bo",
]


def _dev_weights(state, inputs):
    h = hashlib.blake2b(digest_size=16)
    for k in _WEIGHT_KEYS:
        a = np.ascontiguousarray(np.asarray(inputs[k]))
        h.update(a.tobytes())
    key = h.hexdigest()
    if key in _WDEV_CACHE:
        return _WDEV_CACHE[key]
    w = _prep_weights(inputs)
    dev = {}
    for name in state.in_names:
        if name in ("drug", "feature"):
            continue
        g = np.concatenate([w[name]] * N_CORES, axis=0)
        dev[name] = state.jax.device_put(g, state.sharding)
    for a in dev.values():
        a.block_until_ready()
    _WDEV_CACHE[key] = dev
    _WDEV_ORDER.append(key)
    if len(_WDEV_ORDER) > 4:  # bound device memory
        _WDEV_CACHE.pop(_WDEV_ORDER.pop(0), None)
    return dev


def _zeros_dev(state):
    if state.zeros_dev is None:
        state.zeros_dev = [
            state.jax.device_put(
                np.zeros((N_CORES * z.shape[0], *z.shape[1:]), z.dtype),
                state.sharding,
            )
            for z in state.zero_outs
        ]
    return state.zeros_dev


def _make_args(state, wdev, drug, feature):
    args = []
    for name in state.in_names:
        if name == "drug":
            args.append(drug)
        elif name == "feature":
            args.append(feature)
        else:
            args.append(wdev[name])
    args.extend(_zeros_dev(state))
    return args


def _collect(state, out_arrs):
    i = state.out_names.index("out")
    return (
        np.asarray(out_arrs[i]).astype(np.float32, copy=False).reshape(B, 1)
    )


def run(inputs):
    state = _get_state()
    wdev = _dev_weights(state, inputs)
    drug = np.ascontiguousarray(np.asarray(inputs["drug"], np.int32))
    feature = np.ascontiguousarray(np.asarray(inputs["feature"], np.float32))
    out_arrs = state.sharded(*_make_args(state, wdev, drug, feature))
    return _collect(state, out_arrs)


def _bench_one(state, args, iters):
    """Amortized seconds per dispatch: `iters` back-to-back async
    dispatches, single sync at the end."""
    out = state.sharded(*args)  # warm
    for a in out:
        a.block_until_ready()
    prev = None
    t0 = time.perf_counter()
    for _ in range(iters):
        cur = state.sharded(*args)
        del prev
        prev = cur
    for a in prev:
        a.block_until_ready()
    return (time.perf_counter() - t0) / iters, out


def bench_slope(inputs, r_lo=1, r_hi=17, iters=32, rounds=6):
    """Measure device execution time per kernel run via the slope method:
    build two NEFFs that run the full kernel body r_lo / r_hi times
    back-to-back on device, measure amortized wall per dispatch for each,
    and divide the difference by (r_hi - r_lo). Host dispatch and transfer
    overhead are identical for both and cancel, leaving pure device
    execution time per kernel repetition. Measurement noise (tunnel
    jitter) is strictly additive, so each point is measured `rounds`
    times interleaved and the minimum is used.

    Returns (exec_per_rep_s, out_lo, out_hi, t_lo, t_hi)."""
    st_lo = _get_state(r_lo)
    st_hi = _get_state(r_hi)
    wdev = _dev_weights(st_lo, inputs)
    drug = st_lo.jax.device_put(
        np.ascontiguousarray(np.asarray(inputs["drug"], np.int32)), st_lo.sharding
    )
    feature = st_lo.jax.device_put(
        np.ascontiguousarray(np.asarray(inputs["feature"], np.float32)),
        st_lo.sharding,
    )
    args_lo = _make_args(st_lo, wdev, drug, feature)
    args_hi = _make_args(st_hi, wdev, drug, feature)
    ts_lo, ts_hi = [], []
    out_lo = out_hi = None
    for _ in range(rounds):
        t, out_lo = _bench_one(st_lo, args_lo, iters)
        ts_lo.append(t)
        t, out_hi = _bench_one(st_hi, args_hi, iters)
        ts_hi.append(t)
    t_lo, t_hi = min(ts_lo), min(ts_hi)
    exec_per_rep = (t_hi - t_lo) / (r_hi - r_lo)
    return (
        exec_per_rep,
        _collect(st_lo, out_lo),
        _collect(st_hi, out_hi),
        t_lo,
        t_hi,
    )


def kernel(**inputs):
    return run(inputs)


# revision 33
# speedup vs baseline: 1.0086x; 1.0086x over previous
"""AttentionDTI forward on 8 Trainium2 NeuronCores (Bass/Tile), data-parallel.

Layout strategy (per core, batch shard b=256):
  - channels live on SBUF partitions everywhere; positions/samples on free dims
  - embedding lookup fused into conv1: G_k = emb @ dw1[:,:,k].T  (host prep),
    device builds one-hot [65, S*100] from int32 drug ids (broadcast DMA +
    one is_equal tensor_scalar op) and matmuls against G_k
  - conv2/conv3 = shifted matmuls accumulated in PSUM over taps/Cin chunks
  - attention computed channel-major: da = Wda@dc, s = relu(da + fa_bcast),
    A = Watt@s; comp/feat scales via ScalarE sigmoid straight from PSUM
  - both attention matmuls (Wda, Watt) run as fp8e4 DoubleRow (2 contraction
    chunks per instruction): their outputs feed sigmoids / are re-added to
    full-precision paths, so e4m3 quantization is loss-free here (verified
    against an fp32 emulation); dc gets a separate fp8 copy evacuated from
    the same PSUM so the bf16 dc still feeds the (0.5+comp) scale + maxpool
  - MLP (1024-1024-512-1) batched over all 256 samples at the end
Other matmul operands bf16 (PSUM accumulates f32); biases folded into ACT
ops. PSUM banks: 1 conv1 + 1 conv2/out + 6 rotating for conv3/da/attn.

Runner: a process-cached jit(shard_map(bass_exec)) executable with the
replicated weights kept device-resident (keyed by a content hash), so a
steady-state call only ships drug/feature (~860KB) over the axon tunnel
instead of ~70MB of weights.
"""

import hashlib
import sys
import time
from types import SimpleNamespace

if "/opt/trn_rl_repo" not in sys.path:
    sys.path.insert(0, "/opt/trn_rl_repo")

import numpy as np
import ml_dtypes

import concourse.bass as bass
import concourse.tile as tile
from concourse import mybir

BF16 = mybir.dt.bfloat16
F8 = mybir.dt.float8e4
F32 = mybir.dt.float32
I32 = mybir.dt.int32
bf16 = ml_dtypes.bfloat16
f8e4 = ml_dtypes.float8_e4m3  # TRN FP8_EXP4-compatible (max ±240)

N_CORES = 8
B = 2048
B_CORE = B // N_CORES
LD = 100
L1, L2, L3 = 97, 92, 85  # lengths after K=4,6,8 valid convs
S = 5  # samples per tile (S*L1 = 485 <= 512 psum bank; matmul FD>512
#        fails the walrus ISA check -- output must fit one PSUM bank)
PAD8 = 432  # fp8 chunk pitch: >= S*L3=425, multiple of 16 (DoubleRow
#             weight-pair stride constraint)

AF = mybir.ActivationFunctionType
ALU = mybir.AluOpType


# --------------------------------------------------------------------------
# walrus's CTRL codegen handles at most 2 sem waits on one instruction; the
# Tile tail drain can carry many. Split them across single-wait SP nops.
def _patched_drain_and_barrier(self, tick_clock, wait_clock):
    from concourse.tile import ScopedClock

    nc = self.nc
    probe = nc.sync.nop()
    wait_clock.add_sem_waits(probe.ins, ScopedClock({None: tick_clock.global_clock}))
    si = probe.ins.sync_info
    waits = list(si.on_wait) if si is not None else []
    if si is not None:
        probe.ins.sync_info = mybir.SyncInfo(
            on_update=list(si.on_update), on_wait=waits[:1]
        )
    for w in waits[1:]:
        extra = nc.sync.nop()
        extra.ins.sync_info = mybir.SyncInfo(on_update=[], on_wait=[w])
    nc.sync.drain()
    nc.all_engine_barrier()
    popped = nc._tile_sem_poison_stack.pop()
    assert popped is self._sem_poison
    nc.clear_and_free_semaphores(list(self.sems.allocated().values()))
    nc.all_engine_barrier()


tile.TileContext._drain_and_barrier = _patched_drain_and_barrier


# Same walrus limit applies to every engine instruction, and Tile's sem
# assignment can put 3+ waits on one op. Rewrite the serialized BIR: any
# instruction with >2 waits gets same-engine NoOps in front carrying the
# surplus (waits are AND conditions, so hoisting preserves semantics).
_MAX_WAITS = 1
# guard against double-wrapping if this module is ever reloaded
if not getattr(bass.Bass.to_json_bytes, "_wait_split_patched", False):
    _orig_to_json_bytes = bass.Bass.to_json_bytes


def _split_waits_to_json_bytes(self, *a, **k):
    import json as _json

    raw = _orig_to_json_bytes(self, *a, **k)
    j = _json.loads(raw)
    ctr = 0
    changed = False
    for f in j.get("functions", []):
        for bb in f.get("blocks", []):
            out = []
            for ins in bb.get("instructions", []):
                si = ins.get("sync_info")
                waits = (si or {}).get("on_wait", [])
                if len(waits) > _MAX_WAITS:
                    changed = True
                    extra, keep = waits[:-_MAX_WAITS], waits[-_MAX_WAITS:]
                    for i in range(0, len(extra), _MAX_WAITS):
                        ctr += 1
                        out.append({
                            "debug": ins.get("debug"),
                            "engine": ins["engine"],
                            "ins": [],
                            "name": f"I-wsplit-{ctr}",
                            "opcode": "NoOp",
                            "outs": [],
                            "sync_info": {
                                "on_update": [],
                                "on_wait": extra[i : i + _MAX_WAITS],
                            },
                        })
                    si["on_wait"] = keep
                out.append(ins)
            bb["instructions"] = out
    if not changed:
        return raw
    return _json.dumps(j).encode()


if not getattr(bass.Bass.to_json_bytes, "_wait_split_patched", False):
    _split_waits_to_json_bytes._wait_split_patched = True
    bass.Bass.to_json_bytes = _split_waits_to_json_bytes
# --------------------------------------------------------------------------


def _bcast_free(ap, n):
    """Append an innermost stride-0 free dim of size n (broadcast read)."""
    return bass.AP(tensor=ap.tensor, offset=ap.offset, ap=list(ap.ap) + [[0, n]])


def _v3(t8, mc, st):
    """[128, st, L3] view of chunk mc of a flat [128, 4, PAD8] fp8 tile
    (the first S*L3 elements of each chunk are contiguous sample blocks)."""
    a = t8[:, mc]
    return bass.AP(
        tensor=a.tensor, offset=a.offset,
        ap=[list(a.ap[0]), [L3, st], [1, L3]],
    )


BIAS_COLS = [
    ("db1", 1), ("db2", 2), ("db3", 4), ("fb1", 1), ("fb2", 2), ("fb3", 4),
    ("fbias", 4), ("batt", 4), ("b1", 8), ("b2", 8), ("b3", 4), ("bo", 1),
]
N_BIAS = sum(n for _, n in BIAS_COLS)  # 43
COL = {}
_c = 0
for _name, _n in BIAS_COLS:
    COL[_name] = _c
    _c += _n


def build_nc(b_core=B_CORE, reps=1):
    """Build the Bass program. With reps>1 the whole kernel body (including
    the weight/const DMAs from DRAM) is emitted reps times back-to-back —
    used only for timing, so device execution time can be measured as the
    slope between two reps values, independent of host dispatch overhead.
    (This walrus build rejects Tile For_i loop instructions, so the reps
    are unrolled.)"""
    nc = bass.Bass()
    dp = nc.declare_dram_parameter

    drug = dp("drug", [b_core, LD], I32, isOutput=False)
    feature = dp("feature", [b_core, 7], F32, isOutput=False)
    iota65 = dp("iota65", [128, 1], F32, isOutput=False)
    g_w = dp("g_w", [128, 2, 128], BF16, isOutput=False)
    w2 = dp("w2", [128, 6, 256], BF16, isOutput=False)
    w3 = dp("w3", [128, 2, 8, 512], BF16, isOutput=False)
    wda = dp("wda", [128, 4, 512], F8, isOutput=False)
    watt = dp("watt", [128, 4, 512], F8, isOutput=False)
    wfa = dp("wfa", [128, 4, 512], BF16, isOutput=False)
    fw1 = dp("fw1", [7, 128], BF16, isOutput=False)
    fw2 = dp("fw2", [128, 256], BF16, isOutput=False)
    fw3 = dp("fw3", [128, 2, 512], BF16, isOutput=False)
    w1m = dp("w1m", [128, 8, 1024], BF16, isOutput=False)
    w2m = dp("w2m", [128, 8, 1024], BF16, isOutput=False)
    w3m = dp("w3m", [128, 8, 512], BF16, isOutput=False)
    wom = dp("wom", [128, 4], BF16, isOutput=False)
    bias = dp("bias", [128, N_BIAS], F32, isOutput=False)
    out_p = dp("out", [1, b_core], F32, isOutput=True)

    mm = nc.tensor.matmul
    act = nc.scalar.activation

    with tile.TileContext(nc) as tc:
        with (
            tc.tile_pool(name="const", bufs=1) as const,
            tc.tile_pool(name="keep", bufs=1) as keep,
            tc.tile_pool(name="work", bufs=2) as work,
            tc.tile_pool(name="ps1", bufs=1, space="PSUM") as ps1,
            tc.tile_pool(name="ps2", bufs=1, space="PSUM") as ps2,
            tc.tile_pool(name="psb", bufs=6, space="PSUM") as psb,
        ):
          def _whole_body():
            # ---------------- constants (needed-first DMA order) -----------
            iota_sb = const.tile([128, 1], F32)
            nc.sync.dma_start(out=iota_sb, in_=iota65[:, :])
            g_sb = const.tile([128, 2, 128], BF16)
            nc.sync.dma_start(out=g_sb, in_=g_w[:, :, :])
            bias_sb = const.tile([128, N_BIAS], F32)
            nc.sync.dma_start(out=bias_sb, in_=bias[:, :])
            w2_sb = const.tile([128, 6, 256], BF16)
            nc.sync.dma_start(out=w2_sb, in_=w2[:, :, :])
            fw1_sb = const.tile([7, 128], BF16)
            nc.sync.dma_start(out=fw1_sb, in_=fw1[:, :])
            fw2_sb = const.tile([128, 256], BF16)
            nc.sync.dma_start(out=fw2_sb, in_=fw2[:, :])
            fw3_sb = const.tile([128, 2, 512], BF16)
            nc.sync.dma_start(out=fw3_sb, in_=fw3[:, :, :])
            f_sb = keep.tile([7, b_core], F32)
            feat_ap = feature[:, :]
            fT = bass.AP(
                tensor=feat_ap.tensor, offset=feat_ap.offset, ap=[[1, 7], [7, b_core]]
            )
            nc.sync.dma_start(out=f_sb, in_=fT)
            wfa_sb = const.tile([128, 4, 512], BF16)
            nc.sync.dma_start(out=wfa_sb, in_=wfa[:, :, :])
            w3_sb = const.tile([128, 2, 8, 512], BF16)
            nc.sync.dma_start(out=w3_sb, in_=w3[:, :, :, :])
            wda_sb = const.tile([128, 4, 512], F8)
            nc.sync.dma_start(out=wda_sb, in_=wda[:, :, :])
            watt_sb = const.tile([128, 4, 512], F8)
            nc.sync.dma_start(out=watt_sb, in_=watt[:, :, :])
            w1m_sb = const.tile([128, 8, 1024], BF16)
            nc.sync.dma_start(out=w1m_sb, in_=w1m[:, :, :])
            w2m_sb = const.tile([128, 8, 1024], BF16)
            nc.sync.dma_start(out=w2m_sb, in_=w2m[:, :, :])
            w3m_sb = const.tile([128, 8, 512], BF16)
            nc.sync.dma_start(out=w3m_sb, in_=w3m[:, :, :])
            wom_sb = const.tile([128, 4], BF16)
            nc.sync.dma_start(out=wom_sb, in_=wom[:, :])

            def bcol(name, i=0):
                return bias_sb[:, COL[name] + i : COL[name] + i + 1]

            # accumulators for the MLP input
            vd_sb = keep.tile([128, 4, b_core], BF16)
            vf_sb = keep.tile([128, 4, b_core], BF16)

            # ---------------- main per-tile loop (software-pipelined) ------
            # PE program order per steady iteration:
            #   [conv1+conv2](t+1)  [da](t)  [conv3](t+1)  [A](t)
            # so every stage consumes results produced >= one full stage
            # earlier and PE never waits on an evacuation.
            n_tiles = (b_core + S - 1) // S

            def emit_feature_path():
                fb_sb = keep.tile([7, b_core], BF16)
                nc.vector.tensor_copy(out=fb_sb, in_=f_sb)

                psf = ps1.tile([128, b_core], F32, tag="c1")
                mm(psf, lhsT=fw1_sb, rhs=fb_sb, start=True, stop=True)
                h1f = keep.tile([128, b_core], BF16)
                act(out=h1f, in_=psf, func=AF.Relu, bias=bcol("fb1"), scale=1.0)

                h2f = keep.tile([128, 2, b_core], BF16)
                for mc in range(2):
                    psf2 = ps1.tile([128, b_core], F32, tag="c1")
                    mm(psf2, lhsT=fw2_sb[:, mc * 128 : (mc + 1) * 128], rhs=h1f,
                       start=True, stop=True)
                    act(out=h2f[:, mc], in_=psf2, func=AF.Relu, bias=bcol("fb2", mc),
                        scale=1.0)

                fnn_sb = keep.tile([128, 4, b_core], BF16)
                for mc in range(4):
                    psf3 = ps1.tile([128, b_core], F32, tag="c1")
                    for kc in range(2):
                        mm(psf3, lhsT=fw3_sb[:, kc, mc * 128 : (mc + 1) * 128],
                           rhs=h2f[:, kc], start=(kc == 0), stop=(kc == 1))
                    act(out=fnn_sb[:, mc], in_=psf3, func=AF.Relu,
                        bias=bcol("fb3", mc), scale=1.0)

                # fa = Wfa @ featureNN + (bda + bfa)   [512, b] f32, kept
                fa_sb = keep.tile([128, 4, b_core], F32)
                for mc in range(4):
                    psfa = ps1.tile([128, b_core], F32, tag="c1")
                    for kc in range(4):
                        mm(psfa, lhsT=wfa_sb[:, kc, mc * 128 : (mc + 1) * 128],
                           rhs=fnn_sb[:, kc], start=(kc == 0), stop=(kc == 3))
                    nc.vector.tensor_scalar_add(
                        out=fa_sb[:, mc], in0=psfa, scalar1=bcol("fbias", mc)
                    )
                return fnn_sb, fa_sb

            def emit_front(t):
                """drug DMA + packed one-hot + conv1 + conv2 -> h2 tile.
                One-hot rows 0-63 = onehot(v=1..64), rows 64-127 = the same
                shifted left one position (vocab row 0 of the emb-fused conv1
                weight is zero, so it is dropped); conv1 then packs two taps
                into each 128-contract matmul."""
                b0 = t * S
                st = min(S, b_core - b0)
                drug_bc = work.tile([128, S, LD], I32, tag="drug",
                                    name=f"drug_bc{t}")
                drug_ap = drug[:, :]
                src = bass.AP(tensor=drug_ap.tensor,
                              offset=drug_ap.offset + b0 * LD,
                              ap=[[0, 128], [LD, st], [1, LD]])
                nc.gpsimd.dma_start(out=drug_bc[:, :st], in_=src)
                oh = work.tile([128, S, LD], BF16, tag="oh", name=f"oh{t}")
                nc.vector.tensor_scalar(
                    out=oh[0:64, :st], in0=drug_bc[0:64, :st], scalar1=iota_sb[0:64],
                    scalar2=None, op0=ALU.is_equal,
                )
                nc.vector.tensor_scalar(
                    out=oh[64:128, :st, 0 : LD - 1],
                    in0=drug_bc[64:128, :st, 1:LD], scalar1=iota_sb[64:128],
                    scalar2=None, op0=ALU.is_equal,
                )

                pc1 = ps1.tile([128, S, L1], F32, tag="c1", name=f"pc1_{t}")
                for j in range(2):
                    mm(pc1[:, :st], lhsT=g_sb[:, j], rhs=oh[:, :st, 2 * j : 2 * j + L1],
                       start=(j == 0), stop=(j == 1))
                h1 = work.tile([128, S, L1], BF16, tag="h1", name=f"h1_{t}")
                act(out=h1[:, :st], in_=pc1[:, :st], func=AF.Relu, bias=bcol("db1"),
                    scale=1.0)

                h2 = work.tile([128, 2, S, L2], BF16, tag="h2", name=f"h2_{t}")
                for mc in range(2):
                    pc2 = ps2.tile([128, S, L2], F32, tag="c2", name=f"pc2_{t}_{mc}")
                    for k in range(6):
                        mm(pc2[:, :st], lhsT=w2_sb[:, k, mc * 128 : (mc + 1) * 128],
                           rhs=h1[:, :st, k : k + L2], start=(k == 0), stop=(k == 5))
                    act(out=h2[:, mc, :st], in_=pc2[:, :st], func=AF.Relu,
                        bias=bcol("db2", mc), scale=1.0)
                return h2

            def emit_conv3(t, h2):
                b0 = t * S
                st = min(S, b_core - b0)
                dc = work.tile([128, 4, S, L3], BF16, tag="dc", name=f"dc{t}")
                dc8 = work.tile([128, 4, PAD8], F8, tag="dc8", name=f"dc8_{t}")
                for mc in range(4):
                    pc3 = psb.tile([128, S, L3], F32, tag="big", name=f"pc3_{t}_{mc}")
                    i = 0
                    for kc in range(2):
                        for k in range(8):
                            mm(pc3[:, :st],
                               lhsT=w3_sb[:, kc, k, mc * 128 : (mc + 1) * 128],
                               rhs=h2[:, kc, :st, k : k + L3],
                               start=(i == 0), stop=(i == 15))
                            i += 1
                    act(out=dc[:, mc, :st], in_=pc3[:, :st], func=AF.Relu,
                        bias=bcol("db3", mc), scale=1.0)
                    # second evacuation of the same PSUM as an fp8 copy that
                    # only feeds the Wda matmul (DoubleRow needs fp8 operands;
                    # dc itself stays bf16 for the (0.5+comp) scaling + maxpool)
                    act(out=_v3(dc8, mc, st), in_=pc3[:, :st], func=AF.Relu,
                        bias=bcol("db3", mc), scale=1.0)
                return dc, dc8

            def emit_da(t, dc8):
                b0 = t * S
                st = min(S, b_core - b0)
                s8 = work.tile([128, 4, PAD8], F8, tag="s8", name=f"s8_{t}")
                for mc in range(4):
                    pda = psb.tile([128, S, L3], F32, tag="big", name=f"pda_{t}_{mc}")
                    for j in range(2):
                        mm(pda[:, :st],
                           lhsT=wda_sb[:, 2 * j : 2 * j + 2, mc * 128 : (mc + 1) * 128],
                           rhs=dc8[:, 2 * j : 2 * j + 2, : st * L3],
                           perf_mode=mybir.MatmulPerfMode.DoubleRow,
                           start=(j == 0), stop=(j == 1))
                    fa_b = _bcast_free(fa_sb[:, mc, b0 : b0 + st], L3)
                    nc.vector.tensor_tensor(
                        out=_v3(s8, mc, st), in0=pda[:, :st], in1=fa_b, op=ALU.add
                    )
                    nc.vector.tensor_scalar_max(
                        out=_v3(s8, mc, st), in0=_v3(s8, mc, st), scalar1=0.0
                    )
                return s8

            def emit_attn(t, dc, s8):
                b0 = t * S
                st = min(S, b_core - b0)
                dcs = work.tile([128, 4, S, L3], BF16, tag="dcs", name=f"dcs{t}")
                for mc in range(4):
                    pA = psb.tile([128, S, L3], F32, tag="big", name=f"pA_{t}_{mc}")
                    for j in range(2):
                        mm(pA[:, :st],
                           lhsT=watt_sb[:, 2 * j : 2 * j + 2, mc * 128 : (mc + 1) * 128],
                           rhs=s8[:, 2 * j : 2 * j + 2, : st * L3],
                           perf_mode=mybir.MatmulPerfMode.DoubleRow,
                           start=(j == 0), stop=(j == 1))
                    u = work.tile([128, S, L3], BF16, tag="u", name=f"u{t}_{mc}")
                    act(out=u[:, :st], in_=pA[:, :st], func=AF.Sigmoid,
                        bias=bcol("batt", mc), scale=1.0)
                    asum = work.tile([128, S], F32, tag="asum", name=f"as{t}_{mc}")
                    nc.vector.tensor_reduce(
                        out=asum[:, :st], in_=pA[:, :st], axis=mybir.AxisListType.X,
                        op=ALU.add,
                    )
                    fsc = work.tile([128, S], F32, tag="fsc", name=f"fs{t}_{mc}")
                    act(out=fsc[:, :st], in_=asum[:, :st], func=AF.Sigmoid,
                        bias=bcol("batt", mc), scale=1.0 / L3)
                    nc.vector.scalar_tensor_tensor(
                        out=dcs[:, mc, :st], in0=u[:, :st], scalar=0.5,
                        in1=dc[:, mc, :st], op0=ALU.add, op1=ALU.mult,
                    )
                    nc.vector.tensor_reduce(
                        out=vd_sb[:, mc, b0 : b0 + st], in_=dcs[:, mc, :st],
                        axis=mybir.AxisListType.X, op=ALU.max,
                    )
                    nc.vector.scalar_tensor_tensor(
                        out=vf_sb[:, mc, b0 : b0 + st], in0=fsc[:, :st], scalar=0.5,
                        in1=fnn_sb[:, mc, b0 : b0 + st], op0=ALU.add, op1=ALU.mult,
                    )

            h2_cur = emit_front(0)
            fnn_sb, fa_sb = emit_feature_path()
            dc_cur, dc8_cur = emit_conv3(0, h2_cur)
            for t in range(n_tiles):
                h2_next = emit_front(t + 1) if t + 1 < n_tiles else None
                s_cur = emit_da(t, dc8_cur)
                dc_next, dc8_next = (
                    emit_conv3(t + 1, h2_next) if h2_next is not None else (None, None)
                )
                emit_attn(t, dc_cur, s_cur)
                dc_cur, dc8_cur = dc_next, dc8_next

            # ------- MLP over the shard, two batch halves interleaved -------
            def pair(kc):
                return vd_sb[:, kc] if kc < 4 else vf_sb[:, kc - 4]

            def leaky_evac(dst, psm, bias_ap, hb, i):
                z = work.tile([128, b_core // 2], F32, tag="z", name=f"z{hb}_{i}")
                act(out=z, in_=psm, func=AF.Identity, bias=bias_ap, scale=1.0)
                nc.vector.scalar_tensor_tensor(
                    out=dst, in0=z, scalar=0.01, in1=z, op0=ALU.mult, op1=ALU.max
                )

            HB = b_core // 2
            hm1 = keep.tile([128, 8, b_core], BF16)
            hm2 = keep.tile([128, 8, b_core], BF16)
            hm3 = keep.tile([128, 4, b_core], BF16)

            def mlp_layer(wsb, n_mc, rhs_of, dst, bname, hb):
                lo = hb * HB
                sl = slice(lo, lo + HB)
                for mc in range(n_mc):
                    pp, tg = (ps1, "c1") if mc % 2 == 0 else (ps2, "c2")
                    psm = pp.tile([128, HB], F32, tag=tg,
                                  name=f"psm_{bname}_{hb}_{mc}")
                    for kc in range(8):
                        mm(psm, lhsT=wsb[:, kc, mc * 128 : (mc + 1) * 128],
                           rhs=rhs_of(kc)[:, sl], start=(kc == 0), stop=(kc == 7))
                    leaky_evac(dst[:, mc, sl], psm, bcol(bname, mc), hb,
                               f"{bname}{mc}")

            for hb in range(2):
                mlp_layer(w1m_sb, 8, pair, hm1, "b1", hb)
            for hb in range(2):
                mlp_layer(w2m_sb, 8, lambda kc: hm1[:, kc], hm2, "b2", hb)
            for hb in range(2):
                mlp_layer(w3m_sb, 4, lambda kc: hm2[:, kc], hm3, "b3", hb)

            pso = ps2.tile([1, b_core], F32, tag="c2")
            for kc in range(4):
                mm(pso, lhsT=wom_sb[:, kc : kc + 1], rhs=hm3[:, kc],
                   start=(kc == 0), stop=(kc == 3))
            o_sb = work.tile([1, b_core], F32, tag="o")
            nc.vector.tensor_scalar_add(
                out=o_sb, in0=pso, scalar1=bias_sb[0:1, COL["bo"] : COL["bo"] + 1]
            )
            nc.gpsimd.dma_start(out=out_p[:, :], in_=o_sb)

          for _ in range(reps):
              _whole_body()

    return nc


def _prep_weights(inp):
    f32 = np.float32

    def t(x):
        return np.ascontiguousarray(x)

    emb = np.asarray(inp["emb"], f32)
    dw1 = np.asarray(inp["dw1"], f32)
    dw2 = np.asarray(inp["dw2"], f32)
    dw3 = np.asarray(inp["dw3"], f32)
    G = np.stack([emb @ dw1[:, :, k].T for k in range(4)], 0)  # [4, 65, 128]

    w = {}
    iota2 = np.concatenate([np.arange(1, 65), np.arange(1, 65)]).astype(np.float32)
    w["iota65"] = iota2.reshape(128, 1)
    g2 = np.zeros((128, 2, 128), np.float32)
    for j in range(2):
        g2[0:64, j] = G[2 * j][1:65]
        g2[64:128, j] = G[2 * j + 1][1:65]
    w["g_w"] = g2.astype(bf16)
    w["w2"] = t(dw2.transpose(1, 2, 0)).astype(bf16)  # [128, 6, 256]
    w["w3"] = t(
        dw3.reshape(512, 2, 128, 8).transpose(2, 1, 3, 0)
    ).astype(bf16)  # [128, 2, 8, 512]
    for nm, W, dt8 in [("wda", "Wda", f8e4), ("watt", "Watt", f8e4),
                       ("wfa", "Wfa", bf16)]:
        M = np.asarray(inp[W], f32).T  # [c, d]
        w[nm] = t(M.reshape(4, 128, 512).transpose(1, 0, 2)).astype(dt8)
    w["fw1"] = t(np.asarray(inp["fw1"], f32)[:, :, 1].T).astype(bf16)  # [7, 128]
    w["fw2"] = t(np.asarray(inp["fw2"], f32)[:, :, 1].T).astype(bf16)  # [128, 256]
    w["fw3"] = t(
        np.asarray(inp["fw3"], f32)[:, :, 1].T.reshape(2, 128, 512).transpose(1, 0, 2)
    ).astype(bf16)  # [128, 2, 512]
    w["w1m"] = t(
        np.asarray(inp["W1"], f32).T.reshape(8, 128, 1024).transpose(1, 0, 2)
    ).astype(bf16)
    w["w2m"] = t(
        np.asarray(inp["W2"], f32).T.reshape(8, 128, 1024).transpose(1, 0, 2)
    ).astype(bf16)
    w["w3m"] = t(
        np.asarray(inp["W3"], f32).T.reshape(8, 128, 512).transpose(1, 0, 2)
    ).astype(bf16)
    w["wom"] = t(np.asarray(inp["Wo"], f32).T.reshape(4, 128).T).astype(bf16)

    cols = []
    cols.append(np.asarray(inp["db1"], f32).reshape(128, 1))
    cols.append(np.asarray(inp["db2"], f32).reshape(2, 128).T)
    cols.append(np.asarray(inp["db3"], f32).reshape(4, 128).T)
    cols.append(np.asarray(inp["fb1"], f32).reshape(128, 1))
    cols.append(np.asarray(inp["fb2"], f32).reshape(2, 128).T)
    cols.append(np.asarray(inp["fb3"], f32).reshape(4, 128).T)
    fbias = np.asarray(inp["bda"], f32) + np.asarray(inp["bfa"], f32)
    cols.append(fbias.reshape(4, 128).T)
    cols.append(np.asarray(inp["batt"], f32).reshape(4, 128).T)
    cols.append(np.asarray(inp["b1"], f32).reshape(8, 128).T)
    cols.append(np.asarray(inp["b2"], f32).reshape(8, 128).T)
    cols.append(np.asarray(inp["b3"], f32).reshape(4, 128).T)
    bo_val = float(np.asarray(inp["bo"], f32).reshape(-1)[0])
    cols.append(np.full((128, 1), bo_val, f32))
    w["bias"] = np.ascontiguousarray(np.concatenate(cols, axis=1))
    assert w["bias"].shape == (128, N_BIAS)
    return w


# ---------------------------------------------------------------------------
# Runner: build the jit(shard_map(bass_exec)) executable once per process and
# keep the replicated weights device-resident across calls.
# ---------------------------------------------------------------------------

_STATES = {}  # reps -> SimpleNamespace
_WDEV_CACHE = {}  # weights content hash -> dict name -> committed jax.Array
_WDEV_ORDER = []


def _get_state(reps=1):
    if reps in _STATES:
        return _STATES[reps]

    import jax
    from jax.experimental.shard_map import shard_map
    from jax.sharding import Mesh, NamedSharding, PartitionSpec
    from concourse import bass2jax

    bass2jax.install_neuronx_cc_hook()
    nc = build_nc(B_CORE, reps=reps)
    assert nc.dbg_addr is None

    partition_name = (
        nc.partition_id_tensor.name if nc.partition_id_tensor else None
    )
    in_names, out_names, out_avals, zero_outs = [], [], [], []
    for alloc in nc.m.functions[0].allocations:
        if not isinstance(alloc, mybir.MemoryLocationSet):
            continue
        name = alloc.memorylocations[0].name
        if alloc.kind == "ExternalInput":
            if name != partition_name:
                in_names.append(name)
        elif alloc.kind == "ExternalOutput":
            shape = tuple(alloc.tensor_shape)
            dtype = mybir.dt.np(alloc.dtype)
            out_avals.append(jax.core.ShapedArray(shape, dtype))
            out_names.append(name)
            zero_outs.append(np.zeros(shape, dtype))
    n_params = len(in_names)
    n_outs = len(out_names)
    all_in_names = list(in_names) + list(out_names)
    if partition_name is not None:
        all_in_names.append(partition_name)
    # No donation: the kernel writes every element of its outputs, so the
    # zero "output seed" operands are never observed and one shared
    # device-resident zeros array can be reused across calls (donating a
    # fresh host zeros array per call costs ~1.2ms of transfer latency).

    def _body(*args):
        operands = list(args)
        if partition_name is not None:
            operands.append(bass2jax.partition_id_tensor())
        outs = bass2jax._bass_exec_p.bind(
            *operands,
            out_avals=tuple(out_avals),
            in_names=tuple(all_in_names),
            out_names=tuple(out_names),
            lowering_input_output_aliases=(),
            sim_require_finite=True,
            sim_require_nnan=True,
            nc=nc,
        )
        return tuple(outs)

    devices = jax.devices()[:N_CORES]
    assert len(devices) == N_CORES
    mesh = Mesh(np.asarray(devices), ("core",))
    sharding = NamedSharding(mesh, PartitionSpec("core"))
    sharded = jax.jit(
        shard_map(
            _body,
            mesh=mesh,
            in_specs=(PartitionSpec("core"),) * (n_params + n_outs),
            out_specs=(PartitionSpec("core"),) * n_outs,
            check_rep=False,
        ),
        keep_unused=True,
    )
    state = SimpleNamespace(
        nc=nc,
        jax=jax,
        mesh=mesh,
        sharding=sharding,
        sharded=sharded,
        in_names=in_names,
        out_names=out_names,
        zero_outs=zero_outs,
        zeros_dev=None,
    )
    _STATES[reps] = state
    return state


_WEIGHT_KEYS = [
    "emb", "dw1", "db1", "dw2", "db2", "dw3", "db3",
    "fw1", "fb1", "fw2", "fb2", "fw3", "fb3",
    "Wda", "bda", "Wfa", "bfa", "Watt", "batt",
    "W1", "b1", "W2", "b2", "W3", "b3", "Wo", "bo",
]


def _dev_weights(state, inputs):
    h = hashlib.blake2b(digest_size=16)
    for k in _WEIGHT_KEYS:
        a = np.ascontiguousarray(np.asarray(inputs[k]))
        h.update(a.tobytes())
    key = h.hexdigest()
    if key in _WDEV_CACHE:
        return _WDEV_CACHE[key]
    w = _prep_weights(inputs)
    dev = {}
    for name in state.in_names:
        if name in ("drug", "feature"):
            continue
        g = np.concatenate([w[name]] * N_CORES, axis=0)
        dev[name] = state.jax.device_put(g, state.sharding)
    for a in dev.values():
        a.block_until_ready()
    _WDEV_CACHE[key] = dev
    _WDEV_ORDER.append(key)
    if len(_WDEV_ORDER) > 4:  # bound device memory
        _WDEV_CACHE.pop(_WDEV_ORDER.pop(0), None)
    return dev


def _zeros_dev(state):
    if state.zeros_dev is None:
        state.zeros_dev = [
            state.jax.device_put(
                np.zeros((N_CORES * z.shape[0], *z.shape[1:]), z.dtype),
                state.sharding,
            )
            for z in state.zero_outs
        ]
    return state.zeros_dev


def _make_args(state, wdev, drug, feature):
    args = []
    for name in state.in_names:
        if name == "drug":
            args.append(drug)
        elif name == "feature":
            args.append(feature)
        else:
            args.append(wdev[name])
    args.extend(_zeros_dev(state))
    return args


def _collect(state, out_arrs):
    i = state.out_names.index("out")
    return (
        np.asarray(out_arrs[i]).astype(np.float32, copy=False).reshape(B, 1)
    )


def run(inputs):
    state = _get_state()
    wdev = _dev_weights(state, inputs)
    drug = np.ascontiguousarray(np.asarray(inputs["drug"], np.int32))
    feature = np.ascontiguousarray(np.asarray(inputs["feature"], np.float32))
    out_arrs = state.sharded(*_make_args(state, wdev, drug, feature))
    return _collect(state, out_arrs)


def _bench_one(state, args, iters):
    """Amortized seconds per dispatch: `iters` back-to-back async
    dispatches, single sync at the end."""
    out = state.sharded(*args)  # warm
    for a in out:
        a.block_until_ready()
    prev = None
    t0 = time.perf_counter()
    for _ in range(iters):
        cur = state.sharded(*args)
        del prev
        prev = cur
    for a in prev:
        a.block_until_ready()
    return (time.perf_counter() - t0) / iters, out


def bench_slope(inputs, r_lo=1, r_hi=17, iters=12, rounds=8):
    """Measure device execution time per kernel run via the slope method:
    build two NEFFs that run the full kernel body r_lo / r_hi times
    back-to-back on device, measure amortized wall per dispatch for each,
    and divide the difference by (r_hi - r_lo). Host dispatch and transfer
    overhead are identical for both and cancel, leaving pure device
    execution time per kernel repetition.

    Each round measures the lo and hi points back-to-back (they share the
    tunnel's current latency mode) with alternating order (cancels slow
    drift) and a short cooldown (limits the bench's own sustained-load
    throttling); the reported value is the median of the per-round
    differences.

    Returns (exec_per_rep_s, out_lo, out_hi, t_lo, t_hi)."""
    st_lo = _get_state(r_lo)
    st_hi = _get_state(r_hi)
    wdev = _dev_weights(st_lo, inputs)
    drug = st_lo.jax.device_put(
        np.ascontiguousarray(np.asarray(inputs["drug"], np.int32)), st_lo.sharding
    )
    feature = st_lo.jax.device_put(
        np.ascontiguousarray(np.asarray(inputs["feature"], np.float32)),
        st_lo.sharding,
    )
    args_lo = _make_args(st_lo, wdev, drug, feature)
    args_hi = _make_args(st_hi, wdev, drug, feature)
    ts_lo, ts_hi, diffs = [], [], []
    out_lo = out_hi = None
    for r in range(rounds):
        pair = [(st_lo, args_lo, ts_lo), (st_hi, args_hi, ts_hi)]
        if r % 2:
            pair.reverse()
        for st, args, acc in pair:
            t, out = _bench_one(st, args, iters)
            acc.append(t)
            if st is st_lo:
                out_lo = out
            else:
                out_hi = out
            time.sleep(0.25)
        diffs.append(ts_hi[-1] - ts_lo[-1])
    t_lo, t_hi = min(ts_lo), min(ts_hi)
    exec_per_rep = float(np.median(diffs)) / (r_hi - r_lo)
    return (
        exec_per_rep,
        _collect(st_lo, out_lo),
        _collect(st_hi, out_hi),
        t_lo,
        t_hi,
    )


def kernel(**inputs):
    return run(inputs)
